# revision 1
# baseline (speedup 1.0000x reference)
"""Multi-Head Latent Attention (MLA) TRN2 Bass kernel, 8-core tensor-parallel.

Sharding: 16 heads split 2-per-core (tensor parallel). Each core computes
q/kv_up projections for its 2 heads, the full latent projection (replicated),
attention for its (batch, head) pairs, and its partial o_proj contribution;
the host sums the 8 partial outputs.

Dataflow is fully "transposed" so no on-device transposes are needed:
  xT [D, B*S] (host-side transpose) ->
  latentT = Wdown^T xT, qT = Wq^T xT, kvT = Wup^T latentT   (all [*, S])
  kv_nat = latentT^T Wup                                    ([S, *])
  scoresT[keys, q] = kvT^T(slice) qT;  expT = exp(scoresT * scale)
  outT[Dh, q]  = kv_nat^T(slice) expT  (accumulate over key tiles)
  denom[*, q]  = ones^T expT           (accumulate over key tiles)
  outT_norm    = outT * (1/denom);  final = outT_norm^T Wo   ([S, D])
Softmax max-subtraction is skipped: scores are ~N(0, 0.037), |s| < ~1.5.

All matmuls run in float32r (fp32 with 11-bit mantissa, full PE speed).
"""
import sys

sys.path.insert(0, "/opt/trn_rl_repo")

import numpy as np  # noqa: E402

B = 2
S = 2048
D = 2048
H = 16
DH = 128
DL = 512
P = 128
N_CORES = 8
H_LOC = H // N_CORES          # 2 heads per core
HW = H_LOC * DH               # 256: per-core head width
SCALE = float(1.0 / np.sqrt(DH))

D_T = D // P                  # 16 d tiles
L_T = DL // P                 # 4 latent tiles
S_SL = 256                    # projection s-slice width
N_SL = S // S_SL              # 8 slices per batch
QT_W = 512                    # q tile width in attention
N_QT = S // QT_W              # 4
KT = S // P                   # 16 key tiles per batch
ST = S // P                   # 16 s row tiles per batch
NT_W = 512
N_NT = D // NT_W              # 4


def _round_fp32r(x: np.ndarray) -> np.ndarray:
    """Round fp32 to fp32r (11-bit mantissa, RNE), matching device rounding."""
    u = np.ascontiguousarray(x, dtype=np.float32).view(np.uint32)
    low = u & np.uint32(0x00000FFF)
    lsb = (u >> np.uint32(12)) & np.uint32(1)
    round_up = (low > np.uint32(0x800)) | ((low == np.uint32(0x800)) & (lsb == 1))
    out = (u & np.uint32(0xFFFFF000)) + (round_up.astype(np.uint32) << np.uint32(12))
    return out.view(np.float32)


def _build_nc():
    import concourse.tile as tile
    import concourse.mybir as mybir
    from concourse import bacc

    f32 = mybir.dt.float32
    f32r = mybir.dt.float32r
    EXP = mybir.ActivationFunctionType.Exp

    nc = bacc.Bacc("TRN2", target_bir_lowering=False, debug=False)

    xT = nc.dram_tensor("xT", [D, B * S], f32r, kind="ExternalInput").ap()
    wq = nc.dram_tensor("wq", [D, HW], f32r, kind="ExternalInput").ap()
    wdown = nc.dram_tensor("wdown", [D, DL], f32r, kind="ExternalInput").ap()
    wup = nc.dram_tensor("wup", [DL, HW], f32r, kind="ExternalInput").ap()
    wo = nc.dram_tensor("wo", [HW, D], f32r, kind="ExternalInput").ap()
    ones_d = nc.dram_tensor("ones", [P, P], f32r, kind="ExternalInput").ap()
    out = nc.dram_tensor("out", [B * S, D], f32, kind="ExternalOutput").ap()

    with tile.TileContext(nc) as tc:
        with tc.tile_pool(name="w", bufs=1) as wp, \
             tc.tile_pool(name="xs", bufs=2) as xsp, \
             tc.tile_pool(name="big", bufs=1) as bigp, \
             tc.tile_pool(name="sm", bufs=1) as smp, \
             tc.tile_pool(name="ps", bufs=1, space="PSUM") as psp:

            # ---- persistent weights ----
            wq_t = []
            wdown_t = []
            for dt_i in range(D_T):
                t = wp.tile([P, HW], f32r, tag=f"wq_{dt_i}", name=f"wq_{dt_i}")
                nc.sync.dma_start(t[:], wq[dt_i * P:(dt_i + 1) * P, :])
                wq_t.append(t)
                t = wp.tile([P, DL], f32r, tag=f"wd_{dt_i}", name=f"wd_{dt_i}")
                nc.sync.dma_start(t[:], wdown[dt_i * P:(dt_i + 1) * P, :])
                wdown_t.append(t)
            wup_t = []
            for lt in range(L_T):
                t = wp.tile([P, HW], f32r, tag=f"wu_{lt}", name=f"wu_{lt}")
                nc.sync.dma_start(t[:], wup[lt * P:(lt + 1) * P, :])
                wup_t.append(t)
            wo_t = []
            for h in range(H_LOC):
                t = wp.tile([P, D], f32r, tag=f"wo_{h}", name=f"wo_{h}")
                nc.sync.dma_start(t[:], wo[h * P:(h + 1) * P, :])
                wo_t.append(t)
            ones_t = wp.tile([P, P], f32r, tag="ones", name="ones")
            nc.sync.dma_start(ones_t[:], ones_d[:, :])

            for b in range(B):
                col0 = b * S

                # ---- per-batch activation tiles (slots shared across batches)
                latT = [bigp.tile([P, S], f32r, tag=f"latT_{m}", name=f"latT_{b}_{m}")
                        for m in range(L_T)]
                qT = [bigp.tile([P, S], f32r, tag=f"qT_{m}", name=f"qT_{b}_{m}")
                      for m in range(H_LOC)]
                kvT = [bigp.tile([P, S], f32r, tag=f"kvT_{m}", name=f"kvT_{b}_{m}")
                       for m in range(H_LOC)]
                kvn = [bigp.tile([P, HW], f32r, tag=f"kvn_{st}", name=f"kvn_{b}_{st}")
                       for st in range(ST)]

                # ---- Phase A: projections, streamed over s-slices ----
                for j in range(N_SL):
                    sc = col0 + j * S_SL
                    xs = []
                    for dt_i in range(D_T):
                        t = xsp.tile([P, S_SL], f32r, tag=f"xs_{dt_i}",
                                     name=f"xs_{b}_{j}_{dt_i}")
                        nc.sync.dma_start(t[:], xT[dt_i * P:(dt_i + 1) * P,
                                                   sc:sc + S_SL])
                        xs.append(t)
                    # latentT [DL, s-slice]
                    for m in range(L_T):
                        ps = psp.tile([P, S_SL], f32, tag="acc", bufs=2,
                                      name=f"psA_{b}_{j}_l{m}")
                        for dt_i in range(D_T):
                            nc.tensor.matmul(ps[:],
                                             wdown_t[dt_i][:, m * P:(m + 1) * P],
                                             xs[dt_i][:],
                                             start=(dt_i == 0),
                                             stop=(dt_i == D_T - 1))
                        nc.vector.tensor_copy(latT[m][:, j * S_SL:(j + 1) * S_SL],
                                              ps[:])
                    # qT [HW, s-slice]
                    for m in range(H_LOC):
                        ps = psp.tile([P, S_SL], f32, tag="acc", bufs=2,
                                      name=f"psA_{b}_{j}_q{m}")
                        for dt_i in range(D_T):
                            nc.tensor.matmul(ps[:],
                                             wq_t[dt_i][:, m * P:(m + 1) * P],
                                             xs[dt_i][:],
                                             start=(dt_i == 0),
                                             stop=(dt_i == D_T - 1))
                        nc.vector.tensor_copy(qT[m][:, j * S_SL:(j + 1) * S_SL],
                                              ps[:])
                    # kvT [HW, s-slice]
                    for m in range(H_LOC):
                        ps = psp.tile([P, S_SL], f32, tag="acc", bufs=2,
                                      name=f"psA_{b}_{j}_k{m}")
                        for lt in range(L_T):
                            nc.tensor.matmul(ps[:],
                                             wup_t[lt][:, m * P:(m + 1) * P],
                                             latT[lt][:, j * S_SL:(j + 1) * S_SL],
                                             start=(lt == 0),
                                             stop=(lt == L_T - 1))
                        nc.vector.tensor_copy(kvT[m][:, j * S_SL:(j + 1) * S_SL],
                                              ps[:])
                    # kv_nat [s rows, HW]
                    for r_i in range(S_SL // P):
                        st = j * (S_SL // P) + r_i
                        ps = psp.tile([P, HW], f32, tag="acc", bufs=2,
                                      name=f"psA_{b}_{j}_n{r_i}")
                        for lt in range(L_T):
                            nc.tensor.matmul(ps[:],
                                             latT[lt][:, st * P:(st + 1) * P],
                                             wup_t[lt][:, :],
                                             start=(lt == 0),
                                             stop=(lt == L_T - 1))
                        nc.vector.tensor_copy(kvn[st][:], ps[:])

                # ---- Phase B: attention pairs ----
                outT = [bigp.tile([P, S], f32r, tag=f"latT_{m}",
                                  name=f"outT_{b}_{m}")
                        for m in range(H_LOC)]
                for h in range(H_LOC):
                    for qt in range(N_QT):
                        q_sl = slice(qt * QT_W, (qt + 1) * QT_W)
                        ps_o = psp.tile([P, QT_W], f32, tag="po", bufs=2,
                                        name=f"pso_{b}_{h}_{qt}")
                        ps_d = psp.tile([P, QT_W], f32, tag="pd", bufs=2,
                                        name=f"psd_{b}_{h}_{qt}")
                        for kt in range(KT):
                            ps_s = psp.tile([P, QT_W], f32, tag="sc", bufs=2,
                                            name=f"pss_{b}_{h}_{qt}_{kt}")
                            nc.tensor.matmul(ps_s[:],
                                             kvT[h][:, kt * P:(kt + 1) * P],
                                             qT[h][:, q_sl],
                                             start=True, stop=True)
                            e = smp.tile([P, QT_W], f32r, tag="e", bufs=3,
                                         name=f"e_{b}_{h}_{qt}_{kt}")
                            nc.scalar.activation(e[:], ps_s[:], EXP, scale=SCALE)
                            nc.tensor.matmul(ps_o[:],
                                             kvn[kt][:, h * P:(h + 1) * P],
                                             e[:],
                                             start=(kt == 0), stop=(kt == KT - 1))
                            nc.tensor.matmul(ps_d[:], ones_t[:], e[:],
                                             start=(kt == 0), stop=(kt == KT - 1))
                        rcp = smp.tile([P, QT_W], f32, tag="rcp", bufs=2,
                                       name=f"rcp_{b}_{h}_{qt}")
                        nc.vector.reciprocal(rcp[:], ps_d[:])
                        nc.vector.tensor_mul(outT[h][:, q_sl], ps_o[:], rcp[:])

                # ---- Phase C: o_proj partial ----
                for st in range(ST):
                    for nt in range(N_NT):
                        ps = psp.tile([P, NT_W], f32, tag="acc", bufs=2,
                                      name=f"psC_{b}_{st}_{nt}")
                        for h in range(H_LOC):
                            nc.tensor.matmul(ps[:],
                                             outT[h][:, st * P:(st + 1) * P],
                                             wo_t[h][:, nt * NT_W:(nt + 1) * NT_W],
                                             start=(h == 0), stop=(h == H_LOC - 1))
                        fin = smp.tile([P, NT_W], f32, tag="fin", bufs=4,
                                       name=f"fin_{b}_{st}_{nt}")
                        if nt % 2 == 0:
                            nc.vector.tensor_copy(fin[:], ps[:])
                        else:
                            nc.scalar.copy(fin[:], ps[:])
                        nc.gpsimd.dma_start(
                            out[col0 + st * P: col0 + (st + 1) * P,
                                nt * NT_W:(nt + 1) * NT_W],
                            fin[:])

    nc.compile()
    return nc


_NC_CACHE = None


def _get_nc():
    global _NC_CACHE
    if _NC_CACHE is None:
        _NC_CACHE = _build_nc()
    return _NC_CACHE


def _run(x, W_q, W_kv_down, W_kv_up, W_o, trace=False):
    from concourse.bass_utils import run_bass_kernel_spmd

    nc = _get_nc()

    xT = _round_fp32r(
        np.ascontiguousarray(x.reshape(B * S, D).T, dtype=np.float32))
    wdown_r = _round_fp32r(W_kv_down)
    wq_r = _round_fp32r(W_q)
    wup_r = _round_fp32r(W_kv_up)
    wo_r = _round_fp32r(W_o)
    ones = np.ones((P, P), np.float32)

    in_maps = []
    for c in range(N_CORES):
        cs = slice(c * HW, (c + 1) * HW)
        in_maps.append({
            "xT": xT,
            "wq": np.ascontiguousarray(wq_r[:, cs]),
            "wdown": wdown_r,
            "wup": np.ascontiguousarray(wup_r[:, cs]),
            "wo": np.ascontiguousarray(wo_r[cs, :]),
            "ones": ones,
        })

    r = run_bass_kernel_spmd(nc, in_maps, list(range(N_CORES)), trace=trace)
    acc = r.results[0]["out"].astype(np.float64)
    for c in range(1, N_CORES):
        acc += r.results[c]["out"].astype(np.float64)
    return acc.reshape(B, S, D).astype(np.float32), r


def kernel(x, W_q, W_kv_down, W_kv_up, W_o):
    out, _ = _run(x, W_q, W_kv_down, W_kv_up, W_o, trace=False)
    return out
